# revision 20
# baseline (speedup 1.0000x reference)
"""Trainium2 Bass kernel for EnhanceLayerLinear.

Computes out = GroupedLinear(Linear(x)):
    y = x @ W.T + b                      [B,S,D]
    out[..., g, :] = y[..., g, :] @ Wg[g].T + bg[g]   (block-diagonal, G groups)

The two stages fold into ONE dense GEMM: because the grouped stage is a
block-diagonal linear applied to y, we have

    out = x @ W'.T + b'   with   W'[g*128:(g+1)*128, :] = Wg[g] @ W[g*128:(g+1)*128, :]
                                 b' = blockdiag(Wg) @ b + bg

The fold costs 32 small [128x128]@[128x4096] host matmuls (~1.5% of total
FLOPs) and removes the 64 serialized f32r grouped-stage PE slots (the PE is
the bottleneck engine at >93% busy) plus their un-hidable 2-pass fp32
LDWEIGHTS and the end-of-kernel flush chain.

Sharding: data-parallel over tokens (B*S = 8192 -> 1024 per core). Each core
runs the single GEMM stage locally; no collectives.

Mixed precision: the PE streams one moving column per cycle in bf16, but fp8
with perf_mode=DoubleRow packs two contraction rows per cell and streams two
k-tiles per column-cycle. A full-fp8 GEMM misses the 2e-2 error gate, but a
PARTIAL-K split passes: the last M_FP8*2 of the 32 k-tiles run as fp8e4m3
DoubleRow pairs, the rest in bf16 (host-simulated exactly: rel-err 1.46e-2
at M_FP8=4 vs the 2e-2 gate; bf16-only is 1.74e-3). This converts
64 passes x 8 bf16 matmuls (216ns each) into 64 x 4 DR matmuls (~241ns),
~49us/core off the PE roofline.

Scaling: e4m3 has min-normal 2^-6, so raw x (std 1) and W' (std 0.0045)
must be rescaled into range: x_fp8 = e4m3(2^5 x), w_fp8 = e4m3(2^9 W').
Their psum contribution is then 2^14 too large, and psum accumulation cannot
apply a per-part scale -- so the bf16-part weights are pre-scaled by 2^14 as
well (exact in bf16: pure exponent shift) and the single psum accumulator is
evacuated with activation(scale=2^-14, bias=b'), which computes
func(in*scale + bias) in fp32.

Layout trick: y is computed TRANSPOSED (features on partitions, tokens on the
free axis), so each psum tile is one out-group's slice. The host hands the
kernel pre-transposed views of x / W' and re-transposes the output. fp8
operands are pair-packed for DoubleRow: 3D APs [128, 2, cols] where dim1
selects the k-tile of the pair.

Schedule: the first ~30us is DMA-paced, so queue order IS the schedule.
x tiles are [128 x 1024] (full per-core token range, 2KB DMA lines); the
first W' column chunk and the first x tile are queued first so the PE starts
~10us in. Groups 0-3 ramp kt-major-interleaved (8 accumulation groups = all
8 psum banks), paced by the x wave; after the ramp all of x is SBUF-resident
and the remaining 28 groups run og-outer with W' streamed exactly once.
"""

import ml_dtypes
import numpy as np

import concourse.bacc as bacc
import concourse.bass as bass
import concourse.tile as tile
from concourse import mybir
from concourse import bass_utils

f32 = mybir.dt.float32
bf16 = mybir.dt.bfloat16
fp8e4 = mybir.dt.float8e4
ACT_ID = mybir.ActivationFunctionType.Identity
DR = mybir.MatmulPerfMode.DoubleRow

B, S, D = 4, 2048, 4096
T = B * S                 # 8192 tokens
G, IG = 32, 128           # groups x group size (4096 = 32*128)
NCORES = 8
TPC = T // NCORES         # 1024 tokens per core
KT = D // 128             # 32 contraction tiles
M_FP8 = 6                 # fp8 DoubleRow k-tile PAIRS per pass (12 k-tiles)
KTB = KT - 2 * M_FP8      # bf16 k-tiles (24)
KB = KTB * 128            # bf16 contraction width (3072)
NMOV = 512                # moving free dim per matmul (= one psum bank of fp32)
NCH = TPC // NMOV         # 2 token chunks per core
RAMP = 4                  # out-groups interleaved during the DMA-paced ramp
WCHUNK = 1024             # ramp W' column-chunk width (2KB DMA lines)
SX = 2.0 ** 5             # fp8 x scale
SW = 2.0 ** 9             # fp8 W' scale
SOUT = 1.0 / (SX * SW)    # psum evacuation scale (2^-14)

_CACHE = {}


def _build():
    nc = bacc.Bacc("TRN2", target_bir_lowering=False, debug=False)
    # x_d[kt, p, t] = x[core_t0 + t, kt*128 + p]          (xT tiles, 2KB lines)
    # x8_d[j, p, i, t] = e4m3(SX * x[core_t0 + t, (KTB + 2j + i)*128 + p])
    # w_d[og, p, kt*128 + o] = bf16(SX*SW * W'[og*128 + o, kt*128 + p])
    # w8_d[og, p, i, j*128 + o] = e4m3(SW * W'[og*128 + o, (KTB + 2j + i)*128 + p])
    # b_d[i, g] = b'[g*128 + i]
    x_d = nc.dram_tensor("x", [KTB, 128, TPC], bf16, kind="ExternalInput")
    x8_d = nc.dram_tensor("x8", [M_FP8, 128, 2, TPC], fp8e4, kind="ExternalInput")
    w_d = nc.dram_tensor("w", [G, 128, KB], bf16, kind="ExternalInput")
    w8_d = nc.dram_tensor(
        "w8", [G, 128, 2, M_FP8 * 128], fp8e4, kind="ExternalInput"
    )
    b_d = nc.dram_tensor("b", [128, G], f32, kind="ExternalInput")
    # o_d[og, o, t] = out[core_t0 + t, og*128 + o]        (outT)
    o_d = nc.dram_tensor("o", [G, 128, TPC], f32, kind="ExternalOutput")

    with tile.TileContext(nc) as tc:
        with (
            tc.tile_pool(name="xp", bufs=KTB) as xp,
            tc.tile_pool(name="x8p", bufs=M_FP8) as x8p,
            tc.tile_pool(name="wp", bufs=5) as wp,
            tc.tile_pool(name="w8p", bufs=5) as w8p,
            tc.tile_pool(name="cp", bufs=1) as cp,
            tc.tile_pool(name="op", bufs=8) as op,
            tc.tile_pool(name="ps", bufs=8, space=bass.MemorySpace.PSUM) as ps,
        ):
            w_tiles = {}
            w8_tiles = {}

            def load_w(og):
                t = wp.tile([128, KB], bf16, tag="w", name="w")
                nc.sync.dma_start(t[:], w_d[og])
                w_tiles[og] = t
                t8 = w8p.tile([128, 2, M_FP8 * 128], fp8e4, tag="w8", name="w8")
                nc.sync.dma_start(t8[:], w8_d[og])
                w8_tiles[og] = t8

            def chain(acc, w_sb, w8_sb, tch):
                tlo, thi = tch * NMOV, (tch + 1) * NMOV
                for kt in range(KTB):
                    nc.tensor.matmul(
                        acc[:],
                        w_sb[:, kt * 128:(kt + 1) * 128],
                        x_sb[kt][:, tlo:thi],
                        start=(kt == 0),
                        stop=False,
                    )
                for j in range(M_FP8):
                    nc.tensor.matmul(
                        acc[:],
                        w8_sb[:, :, j * 128:(j + 1) * 128],
                        x8_sb[j][:, :, tlo:thi],
                        start=False,
                        stop=(j == M_FP8 - 1),
                        perf_mode=DR,
                    )

            def emit_out(acc, og, tch):
                o_sb = op.tile([128, NMOV], f32, tag="o", name="o_sb")
                nc.scalar.activation(
                    o_sb[:], acc[:], ACT_ID, bias=b_sb[:, og:og + 1], scale=SOUT
                )
                # Issue the store from the Scalar queue: program-order after
                # its ACT, and keeps the Sync queue free for weight streaming.
                nc.scalar.dma_start(
                    o_d[og][:, tch * NMOV:(tch + 1) * NMOV], o_sb[:]
                )

            # --- DMA queue head: the critical path to the first matmul.
            ramp_w = []
            ramp_w8 = []
            for og in range(RAMP):
                t = wp.tile([128, KB], bf16, tag="w", name="w")
                ramp_w.append(t)
                w_tiles[og] = t
                t8 = w8p.tile([128, 2, M_FP8 * 128], fp8e4, tag="w8", name="w8")
                ramp_w8.append(t8)
                w8_tiles[og] = t8
            x_sb = [None] * KTB
            x8_sb = [None] * M_FP8

            def load_x(kt):
                t = xp.tile([128, TPC], bf16, tag="x", name="x_sb")
                nc.gpsimd.dma_start(t[:], x_d[kt])
                x_sb[kt] = t

            # The x stream issues from the (otherwise idle) GpSimd queue and
            # the W' stream from Sync, halving the serialized ~0.7us-per-
            # trigger cost on the ramp critical path. The first pieces are
            # small (W' 256 cols, x 512 tokens) so the first matmul fires as
            # early as possible.
            b_sb = cp.tile([128, G], f32)
            x0 = xp.tile([128, TPC], bf16, tag="x", name="x_sb")
            x_sb[0] = x0
            nc.gpsimd.dma_start(x0[:, 0:NMOV], x_d[0][:, 0:NMOV])
            for og in range(RAMP):
                nc.sync.dma_start(ramp_w[og][:, 0:256], w_d[og][:, 0:256])
            nc.gpsimd.dma_start(x0[:, NMOV:TPC], x_d[0][:, NMOV:TPC])
            for kt in range(1, 8):
                load_x(kt)
            # Ramp W' columns in just-in-time 512-col pieces: piece [lo:lo+512]
            # is needed at kt=lo/128, several microseconds after it lands, and
            # the small pieces keep HBM free for the x stream the PE is
            # actually waiting on.
            wbounds = list(range(256, KB, 512)) + [KB]
            for ci in range(len(wbounds) - 1):
                lo, hi = wbounds[ci], wbounds[ci + 1]
                for og in range(RAMP):
                    nc.sync.dma_start(ramp_w[og][:, lo:hi], w_d[og][:, lo:hi])
                for kt in range(8 + ci * 3, min(11 + ci * 3, KTB)):
                    load_x(kt)
            nc.gpsimd.dma_start(b_sb[:], b_d[:])
            for j in range(M_FP8):
                t8 = x8p.tile([128, 2, TPC], fp8e4, tag="x8", name="x8_sb")
                nc.gpsimd.dma_start(t8[:], x8_d[j])
                x8_sb[j] = t8
            for og in range(RAMP):
                nc.sync.dma_start(ramp_w8[og][:], w8_d[og])
            load_w(RAMP)
            load_w(RAMP + 1)
            load_w(RAMP + 2)

            # --- Ramp: og 0..3 x both token chunks = 8 accumulation groups
            # (all 8 psum banks), advancing kt-major, paced by x arrivals.
            accs = {}
            for og in range(RAMP):
                for t in range(NCH):
                    accs[(og, t)] = ps.tile(
                        [128, NMOV], f32, tag="acc", name="acc"
                    )
            for kt in range(KTB):
                for og in range(RAMP):
                    for t in range(NCH):
                        nc.tensor.matmul(
                            accs[(og, t)][:],
                            ramp_w[og][:, kt * 128:(kt + 1) * 128],
                            x_sb[kt][:, t * NMOV:(t + 1) * NMOV],
                            start=(kt == 0),
                            stop=False,
                        )
            for j in range(M_FP8):
                for og in range(RAMP):
                    for t in range(NCH):
                        nc.tensor.matmul(
                            accs[(og, t)][:],
                            ramp_w8[og][:, :, j * 128:(j + 1) * 128],
                            x8_sb[j][:, :, t * NMOV:(t + 1) * NMOV],
                            start=False,
                            stop=(j == M_FP8 - 1),
                            perf_mode=DR,
                        )
            for og in range(RAMP):
                for t in range(NCH):
                    emit_out(accs.pop((og, t)), og, t)

            # --- Steady state: og-outer, W' streamed once, x resident.
            for og in range(RAMP, G):
                w_sb = w_tiles.pop(og)
                w8_sb = w8_tiles.pop(og)
                if og + 3 < G:
                    load_w(og + 3)
                for tch in range(NCH):
                    if og == G - 1 and tch == NCH - 1:
                        # Final chain: two half-width accumulators, so the
                        # first half's ACT + store overlap the second half's
                        # matmuls instead of serializing after the last MM.
                        for h in range(2):
                            lo = tch * NMOV + h * (NMOV // 2)
                            hi = lo + NMOV // 2
                            acc = ps.tile(
                                [128, NMOV // 2], f32, tag="acc", name="acc"
                            )
                            for kt in range(KTB):
                                nc.tensor.matmul(
                                    acc[:],
                                    w_sb[:, kt * 128:(kt + 1) * 128],
                                    x_sb[kt][:, lo:hi],
                                    start=(kt == 0),
                                    stop=False,
                                )
                            for j in range(M_FP8):
                                nc.tensor.matmul(
                                    acc[:],
                                    w8_sb[:, :, j * 128:(j + 1) * 128],
                                    x8_sb[j][:, :, lo:hi],
                                    start=False,
                                    stop=(j == M_FP8 - 1),
                                    perf_mode=DR,
                                )
                            o_sb = op.tile(
                                [128, NMOV // 2], f32, tag="o", name="o_sb"
                            )
                            nc.scalar.activation(
                                o_sb[:], acc[:], ACT_ID,
                                bias=b_sb[:, og:og + 1], scale=SOUT,
                            )
                            nc.scalar.dma_start(o_d[og][:, lo:hi], o_sb[:])
                    else:
                        acc = ps.tile([128, NMOV], f32, tag="acc", name="acc")
                        chain(acc, w_sb, w8_sb, tch)
                        emit_out(acc, og, tch)

    nc.compile()
    return nc


def _get_nc():
    if "nc" not in _CACHE:
        _CACHE["nc"] = _build()
    return _CACHE["nc"]


def _prep_inputs(x, W, b, Wg, bg):
    x = np.ascontiguousarray(x, dtype=np.float32)
    W = np.ascontiguousarray(W, dtype=np.float32)
    b = np.ascontiguousarray(b, dtype=np.float32)
    Wg = np.ascontiguousarray(Wg, dtype=np.float32)
    bg = np.ascontiguousarray(bg, dtype=np.float32)

    # Fold the block-diagonal grouped stage into the dense weights:
    # W'[g] = Wg[g] @ W[g], b' = blockdiag(Wg) @ b + bg.
    Wf = np.matmul(Wg, W.reshape(G, IG, D)).reshape(D, D)
    bf = (np.matmul(Wg, b.reshape(G, IG, 1)).reshape(G, IG) + bg).reshape(D)

    # x: [B,S,D] -> per-core xT tiles; bf16 part [KTB,128,TPC], fp8 pairs
    # [M_FP8,128,2,TPC] (DoubleRow pair-packed, dim "2" = k-tile of the pair).
    xr = x.reshape(NCORES, TPC, KT, 128)
    x_dev = np.ascontiguousarray(
        xr[:, :, :KTB, :].transpose(0, 2, 3, 1).astype(ml_dtypes.bfloat16)
    )
    x8_dev = np.ascontiguousarray(
        (SX * xr[:, :, KTB:, :])
        .reshape(NCORES, TPC, M_FP8, 2, 128)
        .transpose(0, 2, 4, 3, 1)
        .astype(ml_dtypes.float8_e4m3)
    )
    # W': [D_out, D_in] -> [og, p(k_local), kt*128 + o]; bf16 part pre-scaled
    # by SX*SW (exact exponent shift), fp8 pairs [og, p, i, j*128+o].
    wr = Wf.reshape(G, 128, KT, 128)
    w_dev = np.ascontiguousarray(
        (SX * SW * wr[:, :, :KTB, :])
        .transpose(0, 3, 2, 1)
        .reshape(G, 128, KB)
        .astype(ml_dtypes.bfloat16)
    )
    w8_dev = np.ascontiguousarray(
        (SW * wr[:, :, KTB:, :])
        .reshape(G, 128, M_FP8, 2, 128)
        .transpose(0, 4, 3, 2, 1)
        .reshape(G, 128, 2, M_FP8 * 128)
        .astype(ml_dtypes.float8_e4m3)
    )
    b_dev = np.ascontiguousarray(bf.reshape(G, 128).T.astype(np.float32))

    return [
        {
            "x": x_dev[c],
            "x8": x8_dev[c],
            "w": w_dev,
            "w8": w8_dev,
            "b": b_dev,
        }
        for c in range(NCORES)
    ]


def _run(x, W, b, Wg, bg, trace=False, tmpdir=None):
    in_maps = _prep_inputs(x, W, b, Wg, bg)
    nc = _get_nc()
    res = bass_utils.run_bass_kernel_spmd(
        nc, in_maps, core_ids=list(range(NCORES)), trace=trace, tmpdir=tmpdir
    )
    _CACHE["last_result"] = res

    out_t = np.concatenate(
        [res.results[c]["o"].reshape(D, TPC) for c in range(NCORES)], axis=1
    )
    return np.ascontiguousarray(out_t.T).reshape(B, S, D)


def kernel(x, W, b, Wg, bg):
    return _run(x, W, b, Wg, bg, trace=False)


# revision 22
# speedup vs baseline: 1.1956x; 1.1956x over previous
"""Trainium2 Bass kernel for EnhanceLayerLinear.

Computes out = GroupedLinear(Linear(x)):
    y = x @ W.T + b                      [B,S,D]
    out[..., g, :] = y[..., g, :] @ Wg[g].T + bg[g]   (block-diagonal, G groups)

The two stages fold into ONE dense GEMM: because the grouped stage is a
block-diagonal linear applied to y, we have

    out = x @ W'.T + b'   with   W'[g*128:(g+1)*128, :] = Wg[g] @ W[g*128:(g+1)*128, :]
                                 b' = blockdiag(Wg) @ b + bg

The fold costs 32 small [128x128]@[128x4096] host matmuls (~1.5% of total
FLOPs) and removes the 64 serialized f32r grouped-stage PE slots (the PE is
the bottleneck engine at >93% busy) plus their un-hidable 2-pass fp32
LDWEIGHTS and the end-of-kernel flush chain.

Sharding: data-parallel over tokens (B*S = 8192 -> 1024 per core). Each core
runs the single GEMM stage locally; no collectives.

Mixed precision: the PE streams one moving column per cycle in bf16, but fp8
with perf_mode=DoubleRow packs two contraction rows per cell and streams two
k-tiles per column-cycle. A full-fp8 GEMM misses the 2e-2 error gate, but a
PARTIAL-K split passes: the last M_FP8*2 of the 32 k-tiles run as fp8e4m3
DoubleRow pairs, the rest in bf16 (host-simulated exactly: rel-err 1.46e-2
at M_FP8=4 vs the 2e-2 gate; bf16-only is 1.74e-3). This converts
64 passes x 8 bf16 matmuls (216ns each) into 64 x 4 DR matmuls (~241ns),
~49us/core off the PE roofline.

Scaling: e4m3 has min-normal 2^-6, so raw x (std 1) and W' (std 0.0045)
must be rescaled into range: x_fp8 = e4m3(2^5 x), w_fp8 = e4m3(2^9 W').
Their psum contribution is then 2^14 too large, and psum accumulation cannot
apply a per-part scale -- so the bf16-part weights are pre-scaled by 2^14 as
well (exact in bf16: pure exponent shift) and the single psum accumulator is
evacuated with activation(scale=2^-14, bias=b'), which computes
func(in*scale + bias) in fp32.

Layout trick: y is computed TRANSPOSED (features on partitions, tokens on the
free axis), so each psum tile is one out-group's slice. The host hands the
kernel pre-transposed views of x / W' and re-transposes the output. fp8
operands are pair-packed for DoubleRow: 3D APs [128, 2, cols] where dim1
selects the k-tile of the pair.

Schedule: the first ~30us is DMA-paced, so queue order IS the schedule.
x tiles are [128 x 1024] (full per-core token range, 2KB DMA lines); the
first W' column chunk and the first x tile are queued first so the PE starts
~10us in. Groups 0-3 ramp kt-major-interleaved (8 accumulation groups = all
8 psum banks), paced by the x wave; after the ramp all of x is SBUF-resident
and the remaining 28 groups run og-outer with W' streamed exactly once.
"""

import ml_dtypes
import numpy as np

import concourse.bacc as bacc
import concourse.bass as bass
import concourse.tile as tile
from concourse import mybir
from concourse import bass_utils

f32 = mybir.dt.float32
bf16 = mybir.dt.bfloat16
fp8e4 = mybir.dt.float8e4
ACT_ID = mybir.ActivationFunctionType.Identity
DR = mybir.MatmulPerfMode.DoubleRow

B, S, D = 4, 2048, 4096
T = B * S                 # 8192 tokens
G, IG = 32, 128           # groups x group size (4096 = 32*128)
NCORES = 8
TPC = T // NCORES         # 1024 tokens per core
KT = D // 128             # 32 contraction tiles
M_FP8 = 6                 # fp8 DoubleRow k-tile PAIRS per pass (12 k-tiles)
KTB = KT - 2 * M_FP8      # bf16 k-tiles (24)
KB = KTB * 128            # bf16 contraction width (3072)
NMOV = 512                # moving free dim per matmul (= one psum bank of fp32)
NCH = TPC // NMOV         # 2 token chunks per core
RAMP = 4                  # out-groups interleaved during the DMA-paced ramp
WCHUNK = 1024             # ramp W' column-chunk width (2KB DMA lines)
SX = 2.0 ** 5             # fp8 x scale
SW = 2.0 ** 9             # fp8 W' scale
SOUT = 1.0 / (SX * SW)    # psum evacuation scale (2^-14)

_CACHE = {}


def _build():
    nc = bacc.Bacc("TRN2", target_bir_lowering=False, debug=False)
    # x_d[kt, p, t] = x[core_t0 + t, kt*128 + p]          (xT tiles, 2KB lines)
    # x8_d[j, p, i, t] = e4m3(SX * x[core_t0 + t, (KTB + 2j + i)*128 + p])
    # w_d[og, p, kt*128 + o] = bf16(SX*SW * W'[og*128 + o, kt*128 + p])
    # w8_d[og, p, i, j*128 + o] = e4m3(SW * W'[og*128 + o, (KTB + 2j + i)*128 + p])
    # b_d[i, g] = b'[g*128 + i]
    x_d = nc.dram_tensor("x", [KTB, 128, TPC], bf16, kind="ExternalInput")
    x8_d = nc.dram_tensor("x8", [M_FP8, 128, 2, TPC], fp8e4, kind="ExternalInput")
    w_d = nc.dram_tensor("w", [G, 128, KB], bf16, kind="ExternalInput")
    w8_d = nc.dram_tensor(
        "w8", [G, 128, 2, M_FP8 * 128], fp8e4, kind="ExternalInput"
    )
    b_d = nc.dram_tensor("b", [128, G], f32, kind="ExternalInput")
    # o_d[og, o, t] = out[core_t0 + t, og*128 + o]        (outT)
    o_d = nc.dram_tensor("o", [G, 128, TPC], f32, kind="ExternalOutput")

    with tile.TileContext(nc) as tc:
        with (
            tc.tile_pool(name="xp", bufs=KTB) as xp,
            tc.tile_pool(name="x8p", bufs=M_FP8) as x8p,
            tc.tile_pool(name="wp", bufs=5) as wp,
            tc.tile_pool(name="w8p", bufs=5) as w8p,
            tc.tile_pool(name="cp", bufs=1) as cp,
            tc.tile_pool(name="wup", bufs=1) as wup,
            tc.tile_pool(name="op", bufs=8) as op,
            tc.tile_pool(name="ps", bufs=8, space=bass.MemorySpace.PSUM) as ps,
        ):
            w_tiles = {}
            w8_tiles = {}

            def load_w(og):
                t = wp.tile([128, KB], bf16, tag="w", name="w")
                nc.sync.dma_start(t[:], w_d[og])
                w_tiles[og] = t
                t8 = w8p.tile([128, 2, M_FP8 * 128], fp8e4, tag="w8", name="w8")
                nc.sync.dma_start(t8[:], w8_d[og])
                w8_tiles[og] = t8

            def chain(acc, w_sb, w8_sb, tch):
                tlo, thi = tch * NMOV, (tch + 1) * NMOV
                for kt in range(KTB):
                    nc.tensor.matmul(
                        acc[:],
                        w_sb[:, kt * 128:(kt + 1) * 128],
                        x_sb[kt][:, tlo:thi],
                        start=(kt == 0),
                        stop=False,
                    )
                for j in range(M_FP8):
                    nc.tensor.matmul(
                        acc[:],
                        w8_sb[:, :, j * 128:(j + 1) * 128],
                        x8_sb[j][:, :, tlo:thi],
                        start=False,
                        stop=(j == M_FP8 - 1),
                        perf_mode=DR,
                    )

            def emit_out(acc, og, tch):
                o_sb = op.tile([128, NMOV], f32, tag="o", name="o_sb")
                nc.scalar.activation(
                    o_sb[:], acc[:], ACT_ID, bias=b_sb[:, og:og + 1], scale=SOUT
                )
                # Issue the store from the Scalar queue: program-order after
                # its ACT, and keeps the Sync queue free for weight streaming.
                nc.scalar.dma_start(
                    o_d[og][:, tch * NMOV:(tch + 1) * NMOV], o_sb[:]
                )

            # --- HAM warmup: the PE's activity monitor throttles the clock
            # to 4/8 until it has seen ~3.4us of sustained matmul activity.
            # Seven dummy matmuls on a zeroed tile burn that window during
            # the DMA dead time before the first x/W' tiles land, so the
            # first real matmuls issue at full clock.
            wu = wup.tile([128, NMOV], bf16)
            nc.vector.memset(wu[:], 0.0)
            wu_ps = ps.tile([128, NMOV], f32, tag="acc", name="wu_ps")
            for _ in range(7):
                nc.tensor.matmul(
                    wu_ps[:], wu[:, 0:128], wu[:], start=True, stop=True
                )

            # --- DMA queue head: the critical path to the first matmul.
            ramp_w = []
            ramp_w8 = []
            for og in range(RAMP):
                t = wp.tile([128, KB], bf16, tag="w", name="w")
                ramp_w.append(t)
                w_tiles[og] = t
                t8 = w8p.tile([128, 2, M_FP8 * 128], fp8e4, tag="w8", name="w8")
                ramp_w8.append(t8)
                w8_tiles[og] = t8
            x_sb = [None] * KTB
            x8_sb = [None] * M_FP8

            def load_x(kt):
                t = xp.tile([128, TPC], bf16, tag="x", name="x_sb")
                nc.gpsimd.dma_start(t[:], x_d[kt])
                x_sb[kt] = t

            # The x stream issues from the (otherwise idle) GpSimd queue and
            # the W' stream from Sync, halving the serialized ~0.7us-per-
            # trigger cost on the ramp critical path. The first pieces are
            # small (W' 256 cols, x 512 tokens) so the first matmul fires as
            # early as possible.
            b_sb = cp.tile([128, G], f32)
            x0 = xp.tile([128, TPC], bf16, tag="x", name="x_sb")
            x_sb[0] = x0
            nc.gpsimd.dma_start(x0[:, 0:NMOV], x_d[0][:, 0:NMOV])
            for og in range(RAMP):
                nc.sync.dma_start(ramp_w[og][:, 0:256], w_d[og][:, 0:256])
            nc.gpsimd.dma_start(x0[:, NMOV:TPC], x_d[0][:, NMOV:TPC])
            for kt in range(1, 8):
                load_x(kt)
            # Ramp W' columns in just-in-time 512-col pieces: piece [lo:lo+512]
            # is needed at kt=lo/128, several microseconds after it lands, and
            # the small pieces keep HBM free for the x stream the PE is
            # actually waiting on.
            wbounds = list(range(256, KB, 512)) + [KB]
            for ci in range(len(wbounds) - 1):
                lo, hi = wbounds[ci], wbounds[ci + 1]
                for og in range(RAMP):
                    nc.sync.dma_start(ramp_w[og][:, lo:hi], w_d[og][:, lo:hi])
                for kt in range(8 + ci * 3, min(11 + ci * 3, KTB)):
                    load_x(kt)
            nc.gpsimd.dma_start(b_sb[:], b_d[:])
            for j in range(M_FP8):
                t8 = x8p.tile([128, 2, TPC], fp8e4, tag="x8", name="x8_sb")
                nc.gpsimd.dma_start(t8[:], x8_d[j])
                x8_sb[j] = t8
            for og in range(RAMP):
                nc.sync.dma_start(ramp_w8[og][:], w8_d[og])
            load_w(RAMP)
            load_w(RAMP + 1)
            load_w(RAMP + 2)

            # --- Ramp: og 0..3 x both token chunks = 8 accumulation groups
            # (all 8 psum banks), advancing kt-major, paced by x arrivals.
            accs = {}
            for og in range(RAMP):
                for t in range(NCH):
                    accs[(og, t)] = ps.tile(
                        [128, NMOV], f32, tag="acc", name="acc"
                    )
            for kt in range(KTB):
                for og in range(RAMP):
                    for t in range(NCH):
                        nc.tensor.matmul(
                            accs[(og, t)][:],
                            ramp_w[og][:, kt * 128:(kt + 1) * 128],
                            x_sb[kt][:, t * NMOV:(t + 1) * NMOV],
                            start=(kt == 0),
                            stop=False,
                        )
            for j in range(M_FP8):
                for og in range(RAMP):
                    for t in range(NCH):
                        nc.tensor.matmul(
                            accs[(og, t)][:],
                            ramp_w8[og][:, :, j * 128:(j + 1) * 128],
                            x8_sb[j][:, :, t * NMOV:(t + 1) * NMOV],
                            start=False,
                            stop=(j == M_FP8 - 1),
                            perf_mode=DR,
                        )
            for og in range(RAMP):
                for t in range(NCH):
                    emit_out(accs.pop((og, t)), og, t)

            # --- Steady state: og-outer, W' streamed once, x resident.
            for og in range(RAMP, G):
                w_sb = w_tiles.pop(og)
                w8_sb = w8_tiles.pop(og)
                if og + 3 < G:
                    load_w(og + 3)
                for tch in range(NCH):
                    if og == G - 1 and tch == NCH - 1:
                        # Final chain: two half-width accumulators, so the
                        # first half's ACT + store overlap the second half's
                        # matmuls instead of serializing after the last MM.
                        for h in range(2):
                            lo = tch * NMOV + h * (NMOV // 2)
                            hi = lo + NMOV // 2
                            acc = ps.tile(
                                [128, NMOV // 2], f32, tag="acc", name="acc"
                            )
                            for kt in range(KTB):
                                nc.tensor.matmul(
                                    acc[:],
                                    w_sb[:, kt * 128:(kt + 1) * 128],
                                    x_sb[kt][:, lo:hi],
                                    start=(kt == 0),
                                    stop=False,
                                )
                            for j in range(M_FP8):
                                nc.tensor.matmul(
                                    acc[:],
                                    w8_sb[:, :, j * 128:(j + 1) * 128],
                                    x8_sb[j][:, :, lo:hi],
                                    start=False,
                                    stop=(j == M_FP8 - 1),
                                    perf_mode=DR,
                                )
                            o_sb = op.tile(
                                [128, NMOV // 2], f32, tag="o", name="o_sb"
                            )
                            nc.scalar.activation(
                                o_sb[:], acc[:], ACT_ID,
                                bias=b_sb[:, og:og + 1], scale=SOUT,
                            )
                            nc.scalar.dma_start(o_d[og][:, lo:hi], o_sb[:])
                    else:
                        acc = ps.tile([128, NMOV], f32, tag="acc", name="acc")
                        chain(acc, w_sb, w8_sb, tch)
                        emit_out(acc, og, tch)

    nc.compile()
    return nc


def _get_nc():
    if "nc" not in _CACHE:
        _CACHE["nc"] = _build()
    return _CACHE["nc"]


def _prep_inputs(x, W, b, Wg, bg):
    x = np.ascontiguousarray(x, dtype=np.float32)
    W = np.ascontiguousarray(W, dtype=np.float32)
    b = np.ascontiguousarray(b, dtype=np.float32)
    Wg = np.ascontiguousarray(Wg, dtype=np.float32)
    bg = np.ascontiguousarray(bg, dtype=np.float32)

    # Fold the block-diagonal grouped stage into the dense weights:
    # W'[g] = Wg[g] @ W[g], b' = blockdiag(Wg) @ b + bg.
    Wf = np.matmul(Wg, W.reshape(G, IG, D)).reshape(D, D)
    bf = (np.matmul(Wg, b.reshape(G, IG, 1)).reshape(G, IG) + bg).reshape(D)

    # x: [B,S,D] -> per-core xT tiles; bf16 part [KTB,128,TPC], fp8 pairs
    # [M_FP8,128,2,TPC] (DoubleRow pair-packed, dim "2" = k-tile of the pair).
    xr = x.reshape(NCORES, TPC, KT, 128)
    x_dev = np.ascontiguousarray(
        xr[:, :, :KTB, :].transpose(0, 2, 3, 1).astype(ml_dtypes.bfloat16)
    )
    x8_dev = np.ascontiguousarray(
        (SX * xr[:, :, KTB:, :])
        .reshape(NCORES, TPC, M_FP8, 2, 128)
        .transpose(0, 2, 4, 3, 1)
        .astype(ml_dtypes.float8_e4m3)
    )
    # W': [D_out, D_in] -> [og, p(k_local), kt*128 + o]; bf16 part pre-scaled
    # by SX*SW (exact exponent shift), fp8 pairs [og, p, i, j*128+o].
    wr = Wf.reshape(G, 128, KT, 128)
    w_dev = np.ascontiguousarray(
        (SX * SW * wr[:, :, :KTB, :])
        .transpose(0, 3, 2, 1)
        .reshape(G, 128, KB)
        .astype(ml_dtypes.bfloat16)
    )
    w8_dev = np.ascontiguousarray(
        (SW * wr[:, :, KTB:, :])
        .reshape(G, 128, M_FP8, 2, 128)
        .transpose(0, 4, 3, 2, 1)
        .reshape(G, 128, 2, M_FP8 * 128)
        .astype(ml_dtypes.float8_e4m3)
    )
    b_dev = np.ascontiguousarray(bf.reshape(G, 128).T.astype(np.float32))

    return [
        {
            "x": x_dev[c],
            "x8": x8_dev[c],
            "w": w_dev,
            "w8": w8_dev,
            "b": b_dev,
        }
        for c in range(NCORES)
    ]


def _run(x, W, b, Wg, bg, trace=False, tmpdir=None):
    in_maps = _prep_inputs(x, W, b, Wg, bg)
    nc = _get_nc()
    res = bass_utils.run_bass_kernel_spmd(
        nc, in_maps, core_ids=list(range(NCORES)), trace=trace, tmpdir=tmpdir
    )
    _CACHE["last_result"] = res

    out_t = np.concatenate(
        [res.results[c]["o"].reshape(D, TPC) for c in range(NCORES)], axis=1
    )
    return np.ascontiguousarray(out_t.T).reshape(B, S, D)


def kernel(x, W, b, Wg, bg):
    return _run(x, W, b, Wg, bg, trace=False)
